# revision 1
# baseline (speedup 1.0000x reference)
"""Trainium2 Bass kernel for nn_DAWNLayer (moe_routing).

Sharding: data-parallel over (batch, sequence) across 8 cores — core c
handles batch c//4, query rows (c%4)*512..+512. K/V are computed
redundantly per core for the core's batch (full S=2048). All weights
replicated. Inputs are pre-transposed / pre-laid-out on the host; each
core's xT is cyclically rolled so its own query block is always cols
0:512 (keeps the SPMD program identical across cores; attention is
permutation-invariant along the key axis).

Device algorithm (activations kept transposed [feature, row] so matmul
contractions land on the partition dim):
  xhat = (x-mean)*rstd computed once (LN1/LN2 share stats); the LN1
  affine (g1,b1) is folded into the Q/K/V/Ws1 weights and biases on the
  host, the LN2 affine is applied at the FFN-input eviction.
  Attention: scores computed transposed st[t,q] per head, softmax
  without max-subtraction (scores bounded, exp safe), denominator via a
  ones-column folded into V_ext, division folded into the ctx eviction.
  Router: dense top-8 masked softmax via vector.max + match_replace,
  then info = emb.T @ w_dense as a dense matmul (no gather).
  Attention path in bf16; router tail and FFN in fp32 data with
  float32r matmuls (full PE rate at N>=256).
"""

import os

os.environ.setdefault("MYCRO_LOCAL_CACHE", "1")

import numpy as np
import ml_dtypes

import concourse.bass as bass
import concourse.mybir as mybir
import concourse.tile as tile
from concourse import bacc
from concourse.bass_utils import run_bass_kernel_spmd
from concourse.masks import make_identity

dt = mybir.dt
BF = ml_dtypes.bfloat16

B, S, D = 2, 2048, 1024
DFF = 4096
H, DH = 16, 64
NN, K = 256, 8
EPS = 1e-5
QB = 512           # own query rows per core
NDT = D // 128     # 8 d-tiles
NTT = S // 128     # 16 t row-tiles
NFT = DFF // 128   # 32 dff tiles
NQS = QB // 128    # 4 q subtiles
VW = 65 * H        # 1040 V_ext width
RW = S - QB        # 1536 non-own rows

F32, F32R, BF16 = dt.float32, dt.float32r, dt.bfloat16


def r(ap):
    return ap.bitcast(F32R)


def build_program():
    nc = bacc.Bacc("TRN2", target_bir_lowering=False, debug=False, num_devices=8)

    def din(name, shape, dtype):
        return nc.dram_tensor(name, list(shape), dtype, kind="ExternalInput").ap()

    t = {}
    t["xT"] = din("xT", (D, S), F32R)
    t["wqT"] = din("wqT", (D, D), BF16)
    t["wkT"] = din("wkT", (D, D), BF16)
    t["wvT_ext"] = din("wvT_ext", (D, VW), BF16)
    t["bv_ext"] = din("bv_ext", (1, VW), BF16)
    t["wsT1"] = din("wsT1", (D, D), F32R)
    t["wsT2"] = din("wsT2", (D, D), F32R)
    t["embT"] = din("embT", (D, NN), F32R)
    t["emb"] = din("emb", (NN, D), F32R)
    t["wnpT"] = din("wnpT", (D, D), F32R)
    t["wupT"] = din("wupT", (D, DFF), F32R)
    t["wdownT"] = din("wdownT", (DFF, D), F32R)
    for nm in ("bq_c", "bk_c", "bs_c", "b2np_c", "bdown_c", "g2_c"):
        t[nm] = din(nm, (128, NDT), F32)
    t["bup_c"] = din("bup_c", (128, NFT), F32)
    t["ones_col"] = din("ones_col", (128, 1), F32R)
    t["yT"] = nc.dram_tensor("yT", [D, QB], F32, kind="ExternalOutput").ap()

    with tile.TileContext(nc) as tc:
        emit(tc, t)
    nc.compile()
    return nc


def emit(tc, t):
    from contextlib import ExitStack
    nc = tc.nc
    A = mybir.AluOpType
    AF = mybir.ActivationFunctionType
    X = mybir.AxisListType.X

    est = ExitStack()
    # ---- whole-kernel pools (left side) ----
    cp = est.enter_context(tc.tile_pool(name="consts", bufs=1))
    ap_ = est.enter_context(tc.tile_pool(name="poolA", bufs=1))
    bp = est.enter_context(tc.tile_pool(name="poolB", bufs=1))

    ones_col = cp.tile([128, 1], F32R, tag="ones_col")
    ones_row = cp.tile([1, 128], BF16, tag="ones_row")
    eps_t = cp.tile([1, 1], F32, tag="eps_t")
    nc.vector.memset(eps_t[:], EPS)
    nc.sync.dma_start(ones_col[:], t["ones_col"])
    nc.vector.memset(ones_row[:], 1.0)

    cols = {}
    for nm in ("bq_c", "bk_c", "bs_c", "b2np_c", "bdown_c", "g2_c"):
        cols[nm] = cp.tile([128, NDT], F32, tag=nm, name=nm)
        nc.sync.dma_start(cols[nm][:], t[nm])
    bup_sb = cp.tile([128, NFT], F32, tag="bup")
    nc.sync.dma_start(bup_sb[:], t["bup_c"])
    bv_sb = cp.tile([1, VW], BF16, tag="bv")
    nc.sync.dma_start(bv_sb[:], t["bv_ext"])

    xt_own = ap_.tile([128, NDT * QB], F32R, tag="xt_own")
    xh_f = bp.tile([128, NDT * QB], F32R, tag="xh_f")       # own xhat fp32

    nc.sync.dma_start(
        xt_own[:].rearrange("p (a n) -> p a n", a=NDT),
        t["xT"][:, 0:QB].rearrange("(a p) n -> p a n", p=128))

    # ---- right-side: bf16 xhat (P1 -> P2) ----
    rstk = ExitStack()
    rp1 = rstk.enter_context(tc.tile_pool(name="xhbf", bufs=1, side="right"))
    xho_bf = rp1.tile([128, NDT * QB], BF16, tag="xho_bf")
    xhr_bf = rp1.tile([128, NDT * RW], BF16, tag="xhr_bf")

    def xhsl(i, c):  # xhat bf16 [d-tile i, t-chunk c of 512]
        if c == 0:
            return xho_bf[:, QB * i:QB * i + 512]
        return xhr_bf[:, RW * i + 512 * (c - 1):RW * i + 512 * c]

    def xhrow(i, tt):  # xhat bf16 [d-tile i, 128 rows of row-tile tt]
        if tt < 4:
            return xho_bf[:, QB * i + 128 * tt:QB * i + 128 * (tt + 1)]
        return xhr_bf[:, RW * i + 128 * (tt - 4):RW * i + 128 * (tt - 3)]

    # ============ P1+V: shared LN stats, xhat, V (chunk-pipelined) ========
    attn_stack = ExitStack()
    vxp = attn_stack.enter_context(tc.tile_pool(name="vxp", bufs=1))
    Vx = vxp.tile([128, NTT * VW], BF16, tag="Vx")

    with tc.tile_pool(name="p1sb", bufs=1) as p1, \
         tc.tile_pool(name="rows", bufs=12) as rows, \
         tc.tile_pool(name="bcp", bufs=2) as bcp:
        xt_rest = p1.tile([128, NDT * RW], F32R, tag="xt_rest")
        nc.sync.dma_start(
            xt_rest[:].rearrange("p (a n) -> p a n", a=NDT),
            t["xT"][:, QB:S].rearrange("(a p) n -> p a n", p=128))
        wv_all = p1.tile([128, NDT * VW], BF16, tag="wv_all")
        for c4 in range(4):
            nc.sync.dma_start(
                wv_all[:].rearrange("p (a n) -> p a n", a=NDT)[:, :, 260 * c4:260 * (c4 + 1)],
                t["wvT_ext"][:, 260 * c4:260 * (c4 + 1)].rearrange(
                    "(a p) n -> p a n", p=128))

        def xsl(i, c):
            if c == 0:
                return xt_own[:, QB * i:QB * i + 512]
            return xt_rest[:, RW * i + 512 * (c - 1):RW * i + 512 * c]

        m_cs, v_cs = [], []
        with tc.tile_pool(name="xsqp", bufs=1) as xsqp, \
             tc.tile_pool(name="p1ps", bufs=4, space="PSUM") as ps1:
            sx_ps = [ps1.tile([1, 512], F32, tag="sx", name="sx")
                     for _ in range(4)]
            sxx_ps = [ps1.tile([1, 512], F32, tag="sxx", name="sxx")
                      for _ in range(4)]
            for i in range(NDT):
                xsq = xsqp.tile([128, S], F32R, tag="xsq")
                nc.scalar.activation(xsq[:, 0:QB],
                                     xt_own[:, QB * i:QB * (i + 1)], AF.Square)
                nc.scalar.activation(xsq[:, QB:S],
                                     xt_rest[:, RW * i:RW * (i + 1)], AF.Square)
                for c in range(4):
                    nc.tensor.matmul(sx_ps[c][:], r(ones_col[:]), r(xsl(i, c)),
                                     start=(i == 0), stop=(i == NDT - 1))
                    nc.tensor.matmul(sxx_ps[c][:], r(ones_col[:]),
                                     r(xsq[:, 512 * c:512 * (c + 1)]),
                                     start=(i == 0), stop=(i == NDT - 1))
            for c in range(4):
                m_c = rows.tile([1, 512], F32, tag="rows", name="m_c")
                v_c = rows.tile([1, 512], F32, tag="rows", name="v_c")
                nc.scalar.activation(m_c[:], sx_ps[c][:], AF.Copy, scale=1.0 / D)
                nc.scalar.activation(v_c[:], sxx_ps[c][:], AF.Copy, scale=1.0 / D)
                m_cs.append(m_c)
                v_cs.append(v_c)

        with tc.tile_pool(name="vps", bufs=2, space="PSUM") as vps:
            for c in range(4):
                m_c, v_c = m_cs[c], v_cs[c]
                msq = rows.tile([1, 512], F32, tag="rows", name="msq")
                nc.vector.tensor_mul(msq[:], m_c[:], m_c[:])
                nc.vector.tensor_sub(v_c[:], v_c[:], msq[:])
                sdev = rows.tile([1, 512], F32, tag="rows", name="sdev")
                nc.scalar.activation(sdev[:], v_c[:], AF.Sqrt, bias=eps_t[:])
                rstd_c = rows.tile([1, 512], F32, tag="rows", name="rstd_c")
                nc.vector.reciprocal(rstd_c[:], sdev[:])
                mhat_c = rows.tile([1, 512], F32, tag="rows", name="mhat_c")
                nc.vector.tensor_mul(mhat_c[:], m_c[:], rstd_c[:])
                rstd_bc = bcp.tile([128, 512], F32, tag="rstd_bc", name="rstd_bc")
                mhat_bc = bcp.tile([128, 512], F32, tag="mhat_bc", name="mhat_bc")
                nc.gpsimd.partition_broadcast(rstd_bc[:], rstd_c[:])
                nc.gpsimd.partition_broadcast(mhat_bc[:], mhat_c[:])
                for i in range(NDT):
                    if c == 0:
                        o = xh_f[:, QB * i:QB * (i + 1)]
                        nc.vector.tensor_mul(o, xt_own[:, QB * i:QB * i + 512],
                                             rstd_bc[:])
                        nc.vector.tensor_sub(o, o, mhat_bc[:])
                        nc.vector.tensor_copy(xho_bf[:, QB * i:QB * (i + 1)], o)
                    else:
                        rr = xt_rest[:, RW * i + 512 * (c - 1):RW * i + 512 * c]
                        nc.vector.tensor_mul(rr, rr, rstd_bc[:])
                        nc.vector.tensor_sub(
                            xhr_bf[:, RW * i + 512 * (c - 1):RW * i + 512 * c],
                            rr, mhat_bc[:])
                # V for this chunk's 4 row-tiles (needs only chunk-c xhat)
                for tt in range(4 * c, 4 * (c + 1)):
                    for half in range(2):
                        ps = vps.tile([128, 1024], F32, tag="v", name="psv")
                        for cc in range(2):
                            c4 = 2 * half + cc
                            dst = ps[:, 512 * cc:512 * cc + 260]
                            for i in range(NDT):
                                nc.tensor.matmul(
                                    dst, xhrow(i, tt),
                                    wv_all[:, VW * i + 260 * c4:VW * i + 260 * (c4 + 1)],
                                    start=(i == 0), stop=False)
                            nc.tensor.matmul(dst, ones_row[:],
                                             bv_sb[:, 260 * c4:260 * (c4 + 1)],
                                             start=False, stop=True)
                        src2 = ps[:].rearrange("p (c n) -> p c n", c=2)[:, :, 0:260]
                        dst2 = Vx[:, VW * tt + 520 * half:VW * tt + 520 * (half + 1)]
                        nc.scalar.activation(
                            dst2.rearrange("p (c n) -> p c n", c=2), src2, AF.Copy)

    # ============ P2b: Q, K ============
    c2 = attn_stack.enter_context(tc.tile_pool(name="attn", bufs=1))
    Kt = c2.tile([128, NDT * S], BF16, tag="Kt")
    Qt = c2.tile([128, NDT * QB], BF16, tag="Qt")

    with tc.tile_pool(name="wkq", bufs=3) as wkq, \
         tc.tile_pool(name="kqps", bufs=4, space="PSUM") as kqps:
        for j in range(NDT):
            wq_t = wkq.tile([128, NDT * 128], BF16, tag="wkq", name="wq_t")
            nc.sync.dma_start(
                wq_t[:].rearrange("p (a n) -> p a n", a=NDT),
                t["wqT"][:, 128 * j:128 * (j + 1)].rearrange("(a p) n -> p a n", p=128))
            ps = kqps.tile([128, 512], F32, tag="kq", name="psq")
            for i in range(NDT):
                nc.tensor.matmul(ps[:], wq_t[:, 128 * i:128 * (i + 1)],
                                 xho_bf[:, QB * i:QB * (i + 1)],
                                 start=(i == 0), stop=(i == NDT - 1))
            nc.scalar.activation(Qt[:, QB * j:QB * (j + 1)], ps[:],
                                 AF.Identity, bias=cols["bq_c"][:, j:j + 1])
        for j in range(NDT):
            wk_t = wkq.tile([128, NDT * 128], BF16, tag="wkq", name="wk_t")
            nc.sync.dma_start(
                wk_t[:].rearrange("p (a n) -> p a n", a=NDT),
                t["wkT"][:, 128 * j:128 * (j + 1)].rearrange("(a p) n -> p a n", p=128))
            for c in range(4):
                ps = kqps.tile([128, 512], F32, tag="kq", name="psk")
                for i in range(NDT):
                    nc.tensor.matmul(ps[:], wk_t[:, 128 * i:128 * (i + 1)],
                                     xhsl(i, c),
                                     start=(i == 0), stop=(i == NDT - 1))
                nc.scalar.activation(Kt[:, S * j + 512 * c:S * j + 512 * (c + 1)],
                                     ps[:], AF.Identity,
                                     bias=cols["bk_c"][:, j:j + 1])

    rstk.close()  # free bf16 xhat rest/own

    # ---- right-side: router tensors (P3 -> P9) ----
    re_ = rstk.enter_context(tc.tile_pool(name="router", bufs=1, side="right"))
    ctxT = re_.tile([128, NDT * QB], F32R, tag="ctxT")
    queryT = re_.tile([128, NDT * QB], F32R, tag="queryT")
    snn_sb = re_.tile([128, NQS * NN], F32, tag="snn")
    w_all = re_.tile([128, NQS * NN], F32, tag="w_all")
    wT_sb = re_.tile([128, 2 * QB], F32R, tag="wT")
    infoT = re_.tile([128, NDT * QB], F32R, tag="infoT")

    # ============ P3: attention ============
    with tc.tile_pool(name="expp", bufs=3) as expp, \
         tc.tile_pool(name="denp", bufs=3) as denp, \
         tc.tile_pool(name="stps", bufs=3, space="PSUM") as stps, \
         tc.tile_pool(name="ctxps", bufs=2, space="PSUM") as ctxps:
        # Software-pipelined across groups AND heads: PE runs group g+1's
        # score matmuls while ACT exps group g, so the in-order PE queue
        # never sits waiting on exp before the AV accumulation.
        NG = 8  # score/exp/AV groups of 2 t-tiles per head
        steps = [(h, g) for h in range(H) for g in range(NG)]

        def scores_mm(h, g):
            j, po = h // 2, 64 * (h % 2)
            st = stps.tile([128, 1024], F32, tag="st", name="st")
            for u in range(2):
                tt = 2 * g + u
                nc.tensor.matmul(
                    st[:, 512 * u:512 * (u + 1)],
                    Kt[po:po + 64, S * j + 128 * tt:S * j + 128 * (tt + 1)],
                    Qt[po:po + 64, QB * j:QB * (j + 1)],
                    start=True, stop=True)
            return st

        ctx_ps_of = {}
        eU_of = {}
        st_of = {}
        LOOKAHEAD = 2

        def emit_exp(idx):
            h, g = steps[idx]
            eU = expp.tile([128, 1024], BF16, tag="eU", name="eU")
            nc.scalar.activation(eU[:], st_of.pop(idx)[:], AF.Exp)
            eU_of[idx] = eU

        def emit_av(idx):
            h, g = steps[idx]
            if g == 0:
                ctx_ps_of[h] = ctxps.tile([65, 512], F32, tag="ctx",
                                          name="ctx_ps")
            eU = eU_of.pop(idx)
            for u in range(2):
                tt = 2 * g + u
                nc.tensor.matmul(ctx_ps_of[h][:],
                                 Vx[:, VW * tt + 65 * h:VW * tt + 65 * (h + 1)],
                                 eU[:, 512 * u:512 * (u + 1)],
                                 start=(tt == 0), stop=(tt == NTT - 1))
            if g == NG - 1:
                j, po = h // 2, 64 * (h % 2)
                ctx_ps = ctx_ps_of.pop(h)
                den = denp.tile([1, 512], F32, tag="den", name="den")
                nc.scalar.activation(den[:], ctx_ps[64:65, :], AF.Copy)
                rec = denp.tile([1, 512], F32, tag="rec", name="rec")
                nc.vector.reciprocal(rec[:], den[:])
                rbc = denp.tile([64, 512], F32, tag="rbc", name="rbc")
                nc.gpsimd.partition_broadcast(rbc[:], rec[:])
                nc.vector.tensor_mul(ctxT[po:po + 64, QB * j:QB * (j + 1)],
                                     ctx_ps[0:64, :], rbc[:])

        for idx, (h, g) in enumerate(steps):
            st_of[idx] = scores_mm(h, g)
            emit_exp(idx)
            if idx >= LOOKAHEAD:
                emit_av(idx - LOOKAHEAD)
        for idx in range(len(steps) - LOOKAHEAD, len(steps)):
            emit_av(idx)

    attn_stack.close()  # free Kt/Vx/Qt
    ap2 = est.enter_context(tc.tile_pool(name="poolA2", bufs=1))
    xaugT = ap2.tile([128, NDT * QB], F32R, tag="xaugT")
    ident = ap2.tile([128, 128], F32, tag="ident")
    make_identity(nc, ident[:])

    # ============ P4: query  P5: snn  P6: router  P7: wT  P8: info  P9: xaug
    with tc.tile_pool(name="wsp", bufs=4) as wsp, \
         tc.tile_pool(name="rtp", bufs=2) as rtp, \
         tc.tile_pool(name="aps", bufs=3, space="PSUM") as aps, \
         tc.tile_pool(name="snnps", bufs=2, space="PSUM") as snnps, \
         tc.tile_pool(name="trps", bufs=2, space="PSUM") as trps:
        for j in range(NDT):
            ws1_t = wsp.tile([128, NDT * 128], F32R, tag="ws", name="ws1_t")
            ws2_t = wsp.tile([128, NDT * 128], F32R, tag="ws", name="ws2_t")
            nc.sync.dma_start(
                ws1_t[:].rearrange("p (a n) -> p a n", a=NDT),
                t["wsT1"][:, 128 * j:128 * (j + 1)].rearrange("(a p) n -> p a n", p=128))
            nc.sync.dma_start(
                ws2_t[:].rearrange("p (a n) -> p a n", a=NDT),
                t["wsT2"][:, 128 * j:128 * (j + 1)].rearrange("(a p) n -> p a n", p=128))
            ps = aps.tile([128, 512], F32, tag="a", name="psws")
            for i in range(NDT):
                nc.tensor.matmul(ps[:], r(ws1_t[:, 128 * i:128 * (i + 1)]),
                                 r(xh_f[:, QB * i:QB * (i + 1)]),
                                 start=(i == 0), stop=False)
            for i in range(NDT):
                nc.tensor.matmul(ps[:], r(ws2_t[:, 128 * i:128 * (i + 1)]),
                                 r(ctxT[:, QB * i:QB * (i + 1)]),
                                 start=False, stop=(i == NDT - 1))
            nc.scalar.activation(queryT[:, QB * j:QB * (j + 1)], ps[:],
                                 AF.Identity, bias=cols["bs_c"][:, j:j + 1])

        with tc.tile_pool(name="embp", bufs=1) as embp:
            embT_sb = embp.tile([128, NDT * NN], F32R, tag="embT")
            emb_sb = embp.tile([128, 2 * D], F32R, tag="emb")
            nc.sync.dma_start(
                embT_sb[:].rearrange("p (a n) -> p a n", a=NDT),
                t["embT"][:].rearrange("(a p) n -> p a n", p=128))
            for n in range(2):
                nc.sync.dma_start(emb_sb[:, D * n:D * (n + 1)],
                                  t["emb"][128 * n:128 * (n + 1), :])

            for qs in range(NQS):
                ps = snnps.tile([128, NN], F32, tag="snn", name="pssnn")
                for i in range(NDT):
                    nc.tensor.matmul(
                        ps[:],
                        r(queryT[:, QB * i + 128 * qs:QB * i + 128 * (qs + 1)]),
                        r(embT_sb[:, NN * i:NN * (i + 1)]),
                        start=(i == 0), stop=(i == NDT - 1))
                nc.scalar.activation(snn_sb[:, NN * qs:NN * (qs + 1)], ps[:],
                                     AF.Copy)

            for qs in range(NQS):
                snn = snn_sb[:, NN * qs:NN * (qs + 1)]
                t8 = rtp.tile([128, 8], F32, tag="t8", name="t8")
                nc.vector.max(t8[:], snn)
                nmx = rtp.tile([128, 1], F32, tag="nmx", name="nmx")
                nc.vector.tensor_scalar(nmx[:], t8[:, 0:1], -1.0, None, A.mult)
                snz = rtp.tile([128, NN], F32, tag="snz", name="snz")
                nc.vector.match_replace(out=snz[:], in_to_replace=t8[:],
                                        in_values=snn, imm_value=-1e30)
                e = rtp.tile([128, NN], F32, tag="e", name="e")
                nc.scalar.activation(e[:], snn, AF.Exp, bias=nmx[:])
                mask = rtp.tile([128, NN], F32, tag="mask", name="mask")
                nc.vector.tensor_tensor(mask[:], snn, snz[:], A.not_equal)
                wu = rtp.tile([128, NN], F32, tag="wu", name="wu")
                nc.vector.tensor_mul(wu[:], e[:], mask[:])
                ssum = rtp.tile([128, 1], F32, tag="ssum", name="ssum")
                nc.vector.tensor_reduce(ssum[:], wu[:], X, A.add)
                rcp = rtp.tile([128, 1], F32, tag="rcp", name="rcp")
                nc.vector.reciprocal(rcp[:], ssum[:])
                nc.vector.tensor_scalar(w_all[:, NN * qs:NN * (qs + 1)], wu[:],
                                        rcp[:], None, A.mult)

            for qs in range(NQS):
                for n in range(2):
                    ps = trps.tile([128, 128], F32, tag="tr", name="pstr")
                    nc.tensor.transpose(
                        ps[:],
                        w_all[:, NN * qs + 128 * n:NN * qs + 128 * (n + 1)],
                        ident[:])
                    nc.scalar.activation(
                        wT_sb[:, QB * n + 128 * qs:QB * n + 128 * (qs + 1)],
                        ps[:], AF.Copy)

            for j in range(NDT):
                ps = aps.tile([128, 512], F32, tag="a", name="psinfo")
                for n in range(2):
                    nc.tensor.matmul(
                        ps[:],
                        r(emb_sb[:, D * n + 128 * j:D * n + 128 * (j + 1)]),
                        r(wT_sb[:, QB * n:QB * (n + 1)]),
                        start=(n == 0), stop=(n == 1))
                nc.scalar.activation(infoT[:, QB * j:QB * (j + 1)], ps[:],
                                     AF.Copy)

        for j in range(NDT):
            wnp_t = wsp.tile([128, NDT * 128], F32R, tag="ws", name="wnp_t")
            nc.sync.dma_start(
                wnp_t[:].rearrange("p (a n) -> p a n", a=NDT),
                t["wnpT"][:, 128 * j:128 * (j + 1)].rearrange("(a p) n -> p a n", p=128))
            ps = aps.tile([128, 512], F32, tag="a", name="psnp")
            for i in range(NDT):
                nc.tensor.matmul(ps[:], r(wnp_t[:, 128 * i:128 * (i + 1)]),
                                 r(infoT[:, QB * i:QB * (i + 1)]),
                                 start=(i == 0), stop=(i == NDT - 1))
            xa = xaugT[:, QB * j:QB * (j + 1)]
            # n2 + bnp = xhat*g2 + (b2 + bnp)
            nc.vector.tensor_scalar(xa, xh_f[:, QB * j:QB * (j + 1)],
                                    cols["g2_c"][:, j:j + 1],
                                    cols["b2np_c"][:, j:j + 1], A.mult, A.add)
            nc.vector.tensor_add(xa, ps[:], xa)

    rstk.close()  # free router tensors

    # ============ P10: FFN up (gelu) ============
    with tc.tile_pool(name="hsb", bufs=1) as hp:
        hT = hp.tile([128, NFT * QB], F32R, tag="hT")
        with tc.tile_pool(name="wup", bufs=4) as wupp, \
             tc.tile_pool(name="fps", bufs=2, space="PSUM") as fps:
            for f in range(NFT):
                wup_t = wupp.tile([128, NDT * 128], F32R, tag="wup", name="wup_t")
                nc.sync.dma_start(
                    wup_t[:].rearrange("p (a n) -> p a n", a=NDT),
                    t["wupT"][:, 128 * f:128 * (f + 1)].rearrange("(a p) n -> p a n", p=128))
                ps = fps.tile([128, 512], F32, tag="f", name="psf")
                for i in range(NDT):
                    nc.tensor.matmul(ps[:], r(wup_t[:, 128 * i:128 * (i + 1)]),
                                     r(xaugT[:, QB * i:QB * (i + 1)]),
                                     start=(i == 0), stop=(i == NDT - 1))
                nc.scalar.activation(hT[:, QB * f:QB * (f + 1)], ps[:],
                                     AF.Gelu, bias=bup_sb[:, f:f + 1])

        # ============ P11: FFN down + residual ============
        with tc.tile_pool(name="ysb", bufs=1) as yp, \
             tc.tile_pool(name="wdn", bufs=4) as wdnp, \
             tc.tile_pool(name="ops", bufs=8, space="PSUM") as ops:
            yT_sb = yp.tile([128, NDT * QB], F32, tag="yT")
            out_ps = [ops.tile([128, 512], F32, tag="o", name="o")
                      for _ in range(NDT)]
            for k in range(NFT):
                wdn_t = wdnp.tile([128, NDT * 128], F32R, tag="wdn", name="wdn_t")
                nc.sync.dma_start(wdn_t[:], t["wdownT"][128 * k:128 * (k + 1), :])
                for j in range(NDT):
                    nc.tensor.matmul(out_ps[j][:],
                                     r(wdn_t[:, 128 * j:128 * (j + 1)]),
                                     r(hT[:, QB * k:QB * (k + 1)]),
                                     start=(k == 0), stop=(k == NFT - 1))
            for j in range(NDT):
                nc.vector.scalar_tensor_tensor(
                    yT_sb[:, QB * j:QB * (j + 1)], out_ps[j][:],
                    cols["bdown_c"][:, j:j + 1], xt_own[:, QB * j:QB * (j + 1)],
                    op0=A.add, op1=A.add)
            nc.sync.dma_start(
                t["yT"][:].rearrange("(a p) n -> p a n", p=128),
                yT_sb[:].rearrange("p (a n) -> p a n", a=NDT))

    est.close()


# ---------------- host side ----------------

def prep_shared(inp):
    f = lambda a: np.ascontiguousarray(np.asarray(a, np.float32))
    bf = lambda a: np.ascontiguousarray(np.asarray(a, BF))
    cols8 = lambda v: np.ascontiguousarray(np.asarray(v, np.float32).reshape(NDT, 128).T)
    g1 = f(inp["g1"])
    b1 = f(inp["b1"])
    Wq, Wk, Wv = f(inp["Wq"]), f(inp["Wk"]), f(inp["Wv"])
    W = {}
    # LN1 affine folded into weights: W @ (xhat*g1 + b1) = (W*g1) @ xhat + W@b1
    W["wqT"] = bf((Wq.T * g1[:, None]) * 0.125)
    W["bq_c"] = cols8((f(inp["bq"]) + Wq @ b1) * 0.125)
    W["wkT"] = bf(Wk.T * g1[:, None])
    W["bk_c"] = cols8(f(inp["bk"]) + Wk @ b1)
    WvTg = Wv.T * g1[:, None]
    bv_eff = f(inp["bv"]) + Wv @ b1
    wv_ext = np.zeros((D, VW), np.float32)
    bv_ext = np.zeros((1, VW), np.float32)
    for h in range(H):
        wv_ext[:, 65 * h:65 * h + 64] = WvTg[:, 64 * h:64 * (h + 1)]
        bv_ext[0, 65 * h:65 * h + 64] = bv_eff[64 * h:64 * (h + 1)]
        bv_ext[0, 65 * h + 64] = 1.0
    W["wvT_ext"] = bf(wv_ext)
    W["bv_ext"] = bf(bv_ext)
    Ws = f(inp["Ws"])
    Ws1, Ws2 = Ws[:, :D], Ws[:, D:]
    W["wsT1"] = np.ascontiguousarray(Ws1.T * g1[:, None])
    W["wsT2"] = np.ascontiguousarray(Ws2.T)
    W["bs_c"] = cols8(f(inp["bs"]) + Ws1 @ b1)
    W["embT"] = np.ascontiguousarray(f(inp["neuron_emb"]).T)
    W["emb"] = f(inp["neuron_emb"])
    W["wnpT"] = np.ascontiguousarray(f(inp["Wnp"]).T)
    W["wupT"] = np.ascontiguousarray(f(inp["Wup"]).T)
    W["wdownT"] = np.ascontiguousarray(f(inp["Wdown"]).T)
    W["b2np_c"] = cols8(f(inp["b2"]) + f(inp["bnp"]))
    W["g2_c"] = cols8(inp["g2"])
    W["bdown_c"] = cols8(inp["bdown"])
    W["bup_c"] = np.ascontiguousarray(f(inp["bup"]).reshape(NFT, 128).T)
    W["ones_col"] = np.ones((128, 1), np.float32)
    return W


_NC_CACHE = {}


def get_nc():
    if "nc" not in _NC_CACHE:
        _NC_CACHE["nc"] = build_program()
    return _NC_CACHE["nc"]


def make_in_maps(inputs):
    W = prep_shared(inputs)
    x = np.asarray(inputs["x"], np.float32)
    in_maps = []
    for c in range(8):
        b, qi = c // 4, c % 4
        q0 = qi * QB
        xT = np.ascontiguousarray(x[b].T)
        xTr = np.ascontiguousarray(np.concatenate([xT[:, q0:], xT[:, :q0]], axis=1))
        m = dict(W)
        m["xT"] = xTr
        in_maps.append(m)
    return in_maps


def kernel(**inputs):
    nc = get_nc()
    in_maps = make_in_maps(inputs)
    res = run_bass_kernel_spmd(nc, in_maps, core_ids=list(range(8)))
    x = np.asarray(inputs["x"])
    y = np.zeros((B, S, D), np.float32)
    for c in range(8):
        b, qi = c // 4, c % 4
        y[b, qi * QB:(qi + 1) * QB, :] = res.results[c]["yT"].T
    return y.astype(x.dtype, copy=False)



# revision 9
# speedup vs baseline: 1.0779x; 1.0779x over previous
"""Trainium2 Bass kernel for nn_DAWNLayer (moe_routing).

Sharding: data-parallel over (batch, sequence) across 8 cores — core c
handles batch c//4, query rows (c%4)*512..+512. K/V are computed
redundantly per core for the core's batch (full S=2048). All weights
replicated. Inputs are pre-transposed / pre-laid-out on the host; each
core's xT is cyclically rolled so its own query block is always cols
0:512 (keeps the SPMD program identical across cores; attention is
permutation-invariant along the key axis).

Device algorithm (activations kept transposed [feature, row] so matmul
contractions land on the partition dim):
  Per-512-token-chunk pipeline: LN stats -> xhat -> V+K for that chunk,
  with the next chunk's stats matmuls woven between the dense V/K
  matmuls (keeps the PE array active so the HAM clock gate stays at
  8/8). LN1 affine folded into Q/K/V weights; LN2 affine applied at the
  FFN-input eviction.
  Attention: even/odd heads interleaved so their K=64 score matmuls run
  concurrently in PE row groups 0-1 / 2-3 (tile_position derived from
  base partitions). Softmax without max-subtraction, denominator via a
  ones-column folded into V_ext, division folded into the ctx eviction.
  The attention phase is exp(ACT)-paced; the router-score xhat-half
  matmuls are woven into the PE idle slots.
  Router: scores = [n1,ctx] @ (emb@Ws).T computed directly (the query
  stage is folded into the weights on the host), dense top-8 masked
  softmax via vector.max + match_replace, then
  info = w_dense @ (emb@Wnp.T) as a dense matmul (Wnp folded on host).
  Attention path in bf16; router tail and FFN in fp32 data with
  float32r matmuls (full PE rate at N>=256).
"""

import os

os.environ.setdefault("MYCRO_LOCAL_CACHE", "1")

import numpy as np
import ml_dtypes

import concourse.bass as bass
import concourse.mybir as mybir
import concourse.tile as tile
from concourse import bacc
from concourse.bass_utils import run_bass_kernel_spmd
from concourse.masks import make_identity

dt = mybir.dt
BF = ml_dtypes.bfloat16

B, S, D = 2, 2048, 1024
DFF = 4096
H, DH = 16, 64
NN, K = 256, 8
EPS = 1e-5
QB = 512           # own query rows per core
NDT = D // 128     # 8 d-tiles
NTT = S // 128     # 16 t row-tiles
NFT = DFF // 128   # 32 dff tiles
NQS = QB // 128    # 4 q subtiles
VW = 65 * H        # 1040 V_ext width
RW = S - QB        # 1536 non-own rows

F32, F32R, BF16 = dt.float32, dt.float32r, dt.bfloat16


def r(ap):
    return ap.bitcast(F32R)


def build_program():
    nc = bacc.Bacc("TRN2", target_bir_lowering=False, debug=False, num_devices=8)

    def din(name, shape, dtype):
        return nc.dram_tensor(name, list(shape), dtype, kind="ExternalInput").ap()

    t = {}
    t["xT"] = din("xT", (D, S), F32R)
    t["wqT"] = din("wqT", (D, D), BF16)
    t["wkT"] = din("wkT", (D, D), BF16)
    t["wvT_ext"] = din("wvT_ext", (D, VW), BF16)
    t["bv_ext"] = din("bv_ext", (1, VW), BF16)
    t["w2xT"] = din("w2xT", (D, NN), F32R)      # (emb @ Ws1g).T, g1-folded
    t["w2cT"] = din("w2cT", (D, NN), F32R)      # (emb @ Ws2).T
    t["bias2"] = din("bias2", (1, NN), F32R)    # emb @ (bs + Ws1 @ b1)
    t["embW"] = din("embW", (NN, D), F32R)      # emb @ Wnp.T
    t["wupT"] = din("wupT", (D, DFF), F32R)
    t["wdownT"] = din("wdownT", (DFF, D), F32R)
    for nm in ("bq_c", "bk_c", "b2np_c", "bdown_c", "g2_c"):
        t[nm] = din(nm, (128, NDT), F32)
    t["bup_c"] = din("bup_c", (128, NFT), F32)
    t["ones_col"] = din("ones_col", (128, 1), F32R)
    t["yT"] = nc.dram_tensor("yT", [D, QB], F32, kind="ExternalOutput").ap()

    with tile.TileContext(nc) as tc:
        emit(tc, t)
    nc.compile()
    return nc


def emit(tc, t):
    from contextlib import ExitStack
    nc = tc.nc
    A = mybir.AluOpType
    AF = mybir.ActivationFunctionType
    X = mybir.AxisListType.X

    est = ExitStack()
    # ---- whole-kernel pools (left side) ----
    cp = est.enter_context(tc.tile_pool(name="consts", bufs=1))
    ap_ = est.enter_context(tc.tile_pool(name="poolA", bufs=1))
    bp = est.enter_context(tc.tile_pool(name="poolB", bufs=1))

    ones_col = cp.tile([128, 1], F32R, tag="ones_col")
    ones_row = cp.tile([1, 128], BF16, tag="ones_row")
    ones_row_f = cp.tile([1, 128], F32, tag="ones_row_f")
    eps_t = cp.tile([1, 1], F32, tag="eps_t")
    nc.vector.memset(eps_t[:], EPS)
    nc.sync.dma_start(ones_col[:], t["ones_col"])
    nc.vector.memset(ones_row[:], 1.0)
    nc.vector.memset(ones_row_f[:], 1.0)

    cols = {}
    for nm in ("bq_c", "bk_c", "b2np_c", "bdown_c", "g2_c"):
        cols[nm] = cp.tile([128, NDT], F32, tag=nm, name=nm)
        nc.sync.dma_start(cols[nm][:], t[nm])
    bup_sb = cp.tile([128, NFT], F32, tag="bup")
    nc.sync.dma_start(bup_sb[:], t["bup_c"])
    bv_sb = cp.tile([1, VW], BF16, tag="bv")
    nc.sync.dma_start(bv_sb[:], t["bv_ext"])
    bias2_sb = cp.tile([1, NN], F32R, tag="bias2")
    nc.sync.dma_start(bias2_sb[:], t["bias2"])

    xt_own = ap_.tile([128, NDT * QB], F32R, tag="xt_own")
    xh_f = bp.tile([128, NDT * QB], F32R, tag="xh_f")       # own xhat fp32

    nc.sync.dma_start(
        xt_own[:].rearrange("p (a n) -> p a n", a=NDT),
        t["xT"][:, 0:QB].rearrange("(a p) n -> p a n", p=128))

    # ---- right-side: bf16 xhat (phase A -> end of K) ----
    rstk = ExitStack()
    rp1 = rstk.enter_context(tc.tile_pool(name="xhbf", bufs=1, side="right"))
    xho_bf = rp1.tile([128, NDT * QB], BF16, tag="xho_bf")
    xhr_bf = rp1.tile([128, NDT * RW], BF16, tag="xhr_bf")

    def xhsl(i, c):  # xhat bf16 [d-tile i, t-chunk c of 512]
        if c == 0:
            return xho_bf[:, QB * i:QB * i + 512]
        return xhr_bf[:, RW * i + 512 * (c - 1):RW * i + 512 * c]

    def xhrow(i, tt):  # xhat bf16 [d-tile i, 128 rows of row-tile tt]
        if tt < 4:
            return xho_bf[:, QB * i + 128 * tt:QB * i + 128 * (tt + 1)]
        return xhr_bf[:, RW * i + 128 * (tt - 4):RW * i + 128 * (tt - 3)]

    # ====== Phase A: per-chunk LN stats -> xhat -> V (stats woven) ======
    attn_stack = ExitStack()
    vxp = attn_stack.enter_context(tc.tile_pool(name="vxp", bufs=1))
    Vx = vxp.tile([128, NTT * VW], BF16, tag="Vx")

    with tc.tile_pool(name="p1sb", bufs=1) as p1, \
         tc.tile_pool(name="rows", bufs=12) as rows, \
         tc.tile_pool(name="bcp", bufs=2) as bcp, \
         tc.tile_pool(name="xsqp", bufs=2) as xsqp, \
         tc.tile_pool(name="statps", bufs=1, space="PSUM") as statps, \
         tc.tile_pool(name="vps", bufs=2, space="PSUM") as vps:
        xt_rest = p1.tile([128, NDT * RW], F32R, tag="xt_rest")
        for cc in range(3):
            nc.sync.dma_start(
                xt_rest[:].rearrange("p (a n) -> p a n", a=NDT)[
                    :, :, 512 * cc:512 * (cc + 1)],
                t["xT"][:, QB + 512 * cc:QB + 512 * (cc + 1)].rearrange(
                    "(a p) n -> p a n", p=128))
        wv_all = p1.tile([128, NDT * VW], BF16, tag="wv_all")
        for c4 in range(4):
            nc.sync.dma_start(
                wv_all[:].rearrange("p (a n) -> p a n", a=NDT)[:, :, 260 * c4:260 * (c4 + 1)],
                t["wvT_ext"][:, 260 * c4:260 * (c4 + 1)].rearrange(
                    "(a p) n -> p a n", p=128))

        def xsl(i, c):
            if c == 0:
                return xt_own[:, QB * i:QB * i + 512]
            return xt_rest[:, RW * i + 512 * (c - 1):RW * i + 512 * c]

        # stats(c): one [1,1024] PSUM tile; [0:512]=sum x, [512:1024]=sum x^2
        stat_ps = {}

        def emit_stats(c, i):
            if i == 0:
                stat_ps[c] = statps.tile([1, 1024], F32, tag="stat", name="stat")
            xsq = xsqp.tile([128, 512], F32R, tag="xsq", name="xsq")
            nc.scalar.activation(xsq[:], xsl(i, c), AF.Square)
            nc.tensor.matmul(stat_ps[c][:, 0:512], r(ones_col[:]), r(xsl(i, c)),
                             start=(i == 0), stop=(i == NDT - 1))
            nc.tensor.matmul(stat_ps[c][:, 512:1024], r(ones_col[:]), r(xsq[:]),
                             start=(i == 0), stop=(i == NDT - 1))

        def finish_stats(c):
            # -> rstd_bc, mhat_bc broadcast tiles for this chunk
            ps = stat_ps.pop(c)
            m_c = rows.tile([1, 512], F32, tag="rows", name="m_c")
            v_c = rows.tile([1, 512], F32, tag="rows", name="v_c")
            nc.scalar.activation(m_c[:], ps[:, 0:512], AF.Copy, scale=1.0 / D)
            nc.scalar.activation(v_c[:], ps[:, 512:1024], AF.Copy, scale=1.0 / D)
            msq = rows.tile([1, 512], F32, tag="rows", name="msq")
            nc.vector.tensor_mul(msq[:], m_c[:], m_c[:])
            nc.vector.tensor_sub(v_c[:], v_c[:], msq[:])
            sdev = rows.tile([1, 512], F32, tag="rows", name="sdev")
            nc.scalar.activation(sdev[:], v_c[:], AF.Sqrt, bias=eps_t[:])
            rstd_c = rows.tile([1, 512], F32, tag="rows", name="rstd_c")
            nc.vector.reciprocal(rstd_c[:], sdev[:])
            mhat_c = rows.tile([1, 512], F32, tag="rows", name="mhat_c")
            nc.vector.tensor_mul(mhat_c[:], m_c[:], rstd_c[:])
            rstd_bc = bcp.tile([128, 512], F32, tag="rstd_bc", name="rstd_bc")
            mhat_bc = bcp.tile([128, 512], F32, tag="mhat_bc", name="mhat_bc")
            nc.gpsimd.partition_broadcast(rstd_bc[:], rstd_c[:])
            nc.gpsimd.partition_broadcast(mhat_bc[:], mhat_c[:])
            return rstd_bc, mhat_bc

        def emit_xhat(c, rstd_bc, mhat_bc):
            for i in range(NDT):
                if c == 0:
                    o = xh_f[:, QB * i:QB * (i + 1)]
                    nc.vector.tensor_mul(o, xt_own[:, QB * i:QB * i + 512],
                                         rstd_bc[:])
                    nc.vector.tensor_sub(o, o, mhat_bc[:])
                    nc.vector.tensor_copy(xho_bf[:, QB * i:QB * (i + 1)], o)
                else:
                    rr = xt_rest[:, RW * i + 512 * (c - 1):RW * i + 512 * c]
                    nc.vector.tensor_mul(rr, rr, rstd_bc[:])
                    nc.vector.tensor_sub(
                        xhr_bf[:, RW * i + 512 * (c - 1):RW * i + 512 * c],
                        rr, mhat_bc[:])

        def emit_v_tt(tt):
            for half in range(2):
                ps = vps.tile([128, 1024], F32, tag="v", name="psv")
                for cc in range(2):
                    c4 = 2 * half + cc
                    dst = ps[:, 512 * cc:512 * cc + 260]
                    for i in range(NDT):
                        nc.tensor.matmul(
                            dst, xhrow(i, tt),
                            wv_all[:, VW * i + 260 * c4:VW * i + 260 * (c4 + 1)],
                            start=(i == 0), stop=False)
                    nc.tensor.matmul(dst, ones_row[:],
                                     bv_sb[:, 260 * c4:260 * (c4 + 1)],
                                     start=False, stop=True)
                src2 = ps[:].rearrange("p (c n) -> p c n", c=2)[:, :, 0:260]
                dst2 = Vx[:, VW * tt + 520 * half:VW * tt + 520 * (half + 1)]
                nc.scalar.activation(
                    dst2.rearrange("p (c n) -> p c n", c=2), src2, AF.Copy)

        # chunk 0 stats up-front
        for i in range(NDT):
            emit_stats(0, i)
        for c in range(4):
            rstd_bc, mhat_bc = finish_stats(c)
            emit_xhat(c, rstd_bc, mhat_bc)
            # V for this chunk's 4 row-tiles; weave next chunk's stats
            for k4, tt in enumerate(range(4 * c, 4 * (c + 1))):
                emit_v_tt(tt)
                if c < 3:
                    emit_stats(c + 1, 2 * k4)
                    emit_stats(c + 1, 2 * k4 + 1)

    # ============ K, Q (streamed weights) ============
    c2 = attn_stack.enter_context(tc.tile_pool(name="attn", bufs=1))
    Kt = c2.tile([128, NDT * S], BF16, tag="Kt")
    Qt = c2.tile([128, NDT * QB], BF16, tag="Qt")

    with tc.tile_pool(name="wkq", bufs=3) as wkq, \
         tc.tile_pool(name="kqps", bufs=4, space="PSUM") as kqps:
        for j in range(NDT):
            wq_t = wkq.tile([128, NDT * 128], BF16, tag="wkq", name="wq_t")
            nc.sync.dma_start(
                wq_t[:].rearrange("p (a n) -> p a n", a=NDT),
                t["wqT"][:, 128 * j:128 * (j + 1)].rearrange("(a p) n -> p a n", p=128))
            ps = kqps.tile([128, 512], F32, tag="kq", name="psq")
            for i in range(NDT):
                nc.tensor.matmul(ps[:], wq_t[:, 128 * i:128 * (i + 1)],
                                 xho_bf[:, QB * i:QB * (i + 1)],
                                 start=(i == 0), stop=(i == NDT - 1))
            nc.scalar.activation(Qt[:, QB * j:QB * (j + 1)], ps[:],
                                 AF.Identity, bias=cols["bq_c"][:, j:j + 1])
        for j in range(NDT):
            wk_t = wkq.tile([128, NDT * 128], BF16, tag="wkq", name="wk_t")
            nc.sync.dma_start(
                wk_t[:].rearrange("p (a n) -> p a n", a=NDT),
                t["wkT"][:, 128 * j:128 * (j + 1)].rearrange("(a p) n -> p a n", p=128))
            for c in range(4):
                ps = kqps.tile([128, 512], F32, tag="kq", name="psk")
                for i in range(NDT):
                    nc.tensor.matmul(ps[:], wk_t[:, 128 * i:128 * (i + 1)],
                                     xhsl(i, c),
                                     start=(i == 0), stop=(i == NDT - 1))
                nc.scalar.activation(Kt[:, S * j + 512 * c:S * j + 512 * (c + 1)],
                                     ps[:], AF.Identity,
                                     bias=cols["bk_c"][:, j:j + 1])

    rstk.close()  # free bf16 xhat rest/own

    # ---- right-side: router tensors (attention -> xaug) ----
    re_ = rstk.enter_context(tc.tile_pool(name="router", bufs=1, side="right"))
    ctxT = re_.tile([128, NDT * QB], F32R, tag="ctxT")
    snn_sb = re_.tile([128, NQS * NN], F32, tag="snn")
    w_all = re_.tile([128, NQS * NN], F32, tag="w_all")
    wT_sb = re_.tile([128, 2 * QB], F32R, tag="wT")
    w2x_sb = re_.tile([128, NDT * NN], F32R, tag="w2x")
    w2c_sb = re_.tile([128, NDT * NN], F32R, tag="w2c")
    nc.sync.dma_start(
        w2x_sb[:].rearrange("p (a n) -> p a n", a=NDT),
        t["w2xT"][:].rearrange("(a p) n -> p a n", p=128))
    nc.sync.dma_start(
        w2c_sb[:].rearrange("p (a n) -> p a n", a=NDT),
        t["w2cT"][:].rearrange("(a p) n -> p a n", p=128))

    # ============ Phase B: attention (dual-parity heads) ============
    snn_stack = ExitStack()
    snnps_pool = snn_stack.enter_context(
        tc.tile_pool(name="snnps", bufs=1, space="PSUM"))
    snn_ps = snnps_pool.tile([128, NQS * NN], F32, tag="snnp")

    # snn xhat-half matmuls, woven into attention PE idle slots
    snn_weave = []
    for qs in range(NQS):
        for i in range(NDT):
            def mk(qs=qs, i=i):
                nc.tensor.matmul(
                    snn_ps[:, NN * qs:NN * (qs + 1)],
                    xh_f[:, QB * i + 128 * qs:QB * i + 128 * (qs + 1)].bitcast(F32R),
                    r(w2x_sb[:, NN * i:NN * (i + 1)]),
                    start=(i == 0), stop=False)
            snn_weave.append(mk)
        def mkb(qs=qs):
            nc.tensor.matmul(
                snn_ps[:, NN * qs:NN * (qs + 1)],
                r(ones_row_f[:, 0:128]), bias2_sb[:],
                start=False, stop=False)
        snn_weave.append(mkb)
    snn_wi = [0]

    def weave_snn(n=1):
        for _ in range(n):
            if snn_wi[0] < len(snn_weave):
                snn_weave[snn_wi[0]]()
                snn_wi[0] += 1

    with tc.tile_pool(name="expp", bufs=4) as expp, \
         tc.tile_pool(name="denp", bufs=3) as denp, \
         tc.tile_pool(name="stps", bufs=2, space="PSUM") as stps, \
         tc.tile_pool(name="ctxps", bufs=2, space="PSUM") as ctxps:
        NG = 8  # score/exp/AV groups of 2 t-tiles per head pair
        steps = [(hp, g) for hp in range(NDT) for g in range(NG)]

        st_of = {}
        eU_of = {}
        ctx_ps_of = {}

        def emit_scores(idx):
            hp, g = steps[idx]
            st_e = stps.tile([128, 1024], F32, tag="st", name="st_e")
            st_o = stps.tile([128, 1024], F32, tag="st", name="st_o")
            for u in range(2):
                tt = 2 * g + u
                ksl = Kt[:, S * hp + 128 * tt:S * hp + 128 * (tt + 1)]
                qsl = Qt[:, QB * hp:QB * (hp + 1)]
                nc.tensor.matmul(st_e[:, 512 * u:512 * (u + 1)],
                                 ksl[0:64, :], qsl[0:64, :],
                                 start=True, stop=True)
                nc.tensor.matmul(st_o[:, 512 * u:512 * (u + 1)],
                                 ksl[64:128, :], qsl[64:128, :],
                                 start=True, stop=True)
            st_of[idx] = (st_e, st_o)

        def emit_exp(idx):
            st_e, st_o = st_of.pop(idx)
            eU_e = expp.tile([128, 1024], BF16, tag="eU", name="eU_e")
            eU_o = expp.tile([128, 1024], BF16, tag="eU", name="eU_o")
            nc.scalar.activation(eU_e[:], st_e[:], AF.Exp)
            nc.scalar.activation(eU_o[:], st_o[:], AF.Exp)
            eU_of[idx] = (eU_e, eU_o)

        def evict_ctx(h, ctx_ps):
            j, po = h // 2, 64 * (h % 2)
            den = denp.tile([1, 512], F32, tag="den", name="den")
            nc.scalar.activation(den[:], ctx_ps[64:65, :], AF.Copy)
            rec = denp.tile([1, 512], F32, tag="rec", name="rec")
            nc.vector.reciprocal(rec[:], den[:])
            rbc = denp.tile([64, 512], F32, tag="rbc", name="rbc")
            nc.gpsimd.partition_broadcast(rbc[:], rec[:])
            nc.vector.tensor_mul(ctxT[po:po + 64, QB * j:QB * (j + 1)],
                                 ctx_ps[0:64, :], rbc[:])

        def emit_av(idx):
            hp, g = steps[idx]
            if g == 0:
                ctx_ps_of[(hp, 0)] = ctxps.tile([65, 512], F32, tag="ctx",
                                                name="ctx_e")
                ctx_ps_of[(hp, 1)] = ctxps.tile([65, 512], F32, tag="ctx",
                                                name="ctx_o")
            eU_e, eU_o = eU_of.pop(idx)
            for u in range(2):
                tt = 2 * g + u
                h_e, h_o = 2 * hp, 2 * hp + 1
                nc.tensor.matmul(ctx_ps_of[(hp, 0)][:],
                                 Vx[:, VW * tt + 65 * h_e:VW * tt + 65 * (h_e + 1)],
                                 eU_e[:, 512 * u:512 * (u + 1)],
                                 start=(tt == 0), stop=(tt == NTT - 1))
                nc.tensor.matmul(ctx_ps_of[(hp, 1)][:],
                                 Vx[:, VW * tt + 65 * h_o:VW * tt + 65 * (h_o + 1)],
                                 eU_o[:, 512 * u:512 * (u + 1)],
                                 start=(tt == 0), stop=(tt == NTT - 1))
            if g == NG - 1:
                evict_ctx(2 * hp, ctx_ps_of.pop((hp, 0)))
                evict_ctx(2 * hp + 1, ctx_ps_of.pop((hp, 1)))

        for idx in range(len(steps)):
            emit_scores(idx)
            emit_exp(idx)
            weave_snn(1)
            if idx >= 1:
                emit_av(idx - 1)
        emit_av(len(steps) - 1)

    attn_stack.close()  # free Kt/Vx/Qt
    ap2 = est.enter_context(tc.tile_pool(name="poolA2", bufs=1))
    xaugT = ap2.tile([128, NDT * QB], F32R, tag="xaugT")
    ident = ap2.tile([128, 128], F32, tag="ident")
    make_identity(nc, ident[:])

    # ======= snn ctx-half =======
    for qs in range(NQS):
        for i in range(NDT):
            nc.tensor.matmul(
                snn_ps[:, NN * qs:NN * (qs + 1)],
                r(ctxT[:, QB * i + 128 * qs:QB * i + 128 * (qs + 1)]),
                r(w2c_sb[:, NN * i:NN * (i + 1)]),
                start=False, stop=(i == NDT - 1))
        nc.scalar.activation(snn_sb[:, NN * qs:NN * (qs + 1)],
                             snn_ps[:, NN * qs:NN * (qs + 1)], AF.Copy)

    snn_stack.close()

    # ======= router top-8, info, xaug =======
    with tc.tile_pool(name="rtp", bufs=2) as rtp, \
         tc.tile_pool(name="aps", bufs=3, space="PSUM") as aps, \
         tc.tile_pool(name="trps", bufs=2, space="PSUM") as trps:
        for qs in range(NQS):
            snn = snn_sb[:, NN * qs:NN * (qs + 1)]
            t8 = rtp.tile([128, 8], F32, tag="t8", name="t8")
            nc.vector.max(t8[:], snn)
            nmx = rtp.tile([128, 1], F32, tag="nmx", name="nmx")
            nc.vector.tensor_scalar(nmx[:], t8[:, 0:1], -1.0, None, A.mult)
            snz = rtp.tile([128, NN], F32, tag="snz", name="snz")
            nc.vector.match_replace(out=snz[:], in_to_replace=t8[:],
                                    in_values=snn, imm_value=-1e30)
            e = rtp.tile([128, NN], F32, tag="e", name="e")
            nc.scalar.activation(e[:], snn, AF.Exp, bias=nmx[:])
            mask = rtp.tile([128, NN], F32, tag="mask", name="mask")
            nc.vector.tensor_tensor(mask[:], snn, snz[:], A.not_equal)
            wu = rtp.tile([128, NN], F32, tag="wu", name="wu")
            nc.vector.tensor_mul(wu[:], e[:], mask[:])
            ssum = rtp.tile([128, 1], F32, tag="ssum", name="ssum")
            nc.vector.tensor_reduce(ssum[:], wu[:], X, A.add)
            rcp = rtp.tile([128, 1], F32, tag="rcp", name="rcp")
            nc.vector.reciprocal(rcp[:], ssum[:])
            nc.vector.tensor_scalar(w_all[:, NN * qs:NN * (qs + 1)], wu[:],
                                    rcp[:], None, A.mult)

        for qs in range(NQS):
            for n in range(2):
                ps = trps.tile([128, 128], F32, tag="tr", name="pstr")
                nc.tensor.transpose(
                    ps[:],
                    w_all[:, NN * qs + 128 * n:NN * qs + 128 * (n + 1)],
                    ident[:])
                nc.scalar.activation(
                    wT_sb[:, QB * n + 128 * qs:QB * n + 128 * (qs + 1)],
                    ps[:], AF.Copy)

        with tc.tile_pool(name="embp", bufs=1) as embp:
            emb_sb = embp.tile([128, 2 * D], F32R, tag="emb")
            for n in range(2):
                nc.sync.dma_start(emb_sb[:, D * n:D * (n + 1)],
                                  t["embW"][128 * n:128 * (n + 1), :])
            for j in range(NDT):
                ps = aps.tile([128, 512], F32, tag="a", name="psinfo")
                for n in range(2):
                    nc.tensor.matmul(
                        ps[:],
                        r(emb_sb[:, D * n + 128 * j:D * n + 128 * (j + 1)]),
                        r(wT_sb[:, QB * n:QB * (n + 1)]),
                        start=(n == 0), stop=(n == 1))
                xa = xaugT[:, QB * j:QB * (j + 1)]
                # n2 + bnp = xhat*g2 + (b2 + bnp)
                nc.vector.tensor_scalar(xa, xh_f[:, QB * j:QB * (j + 1)],
                                        cols["g2_c"][:, j:j + 1],
                                        cols["b2np_c"][:, j:j + 1], A.mult, A.add)
                nc.vector.tensor_add(xa, ps[:], xa)

    rstk.close()  # free router tensors

    # ============ FFN up (gelu) ============
    with tc.tile_pool(name="hsb", bufs=1) as hp:
        hT = hp.tile([128, NFT * QB], F32R, tag="hT")
        with tc.tile_pool(name="wup", bufs=4) as wupp, \
             tc.tile_pool(name="fps", bufs=2, space="PSUM") as fps:
            for f in range(NFT):
                wup_t = wupp.tile([128, NDT * 128], F32R, tag="wup", name="wup_t")
                nc.sync.dma_start(
                    wup_t[:].rearrange("p (a n) -> p a n", a=NDT),
                    t["wupT"][:, 128 * f:128 * (f + 1)].rearrange("(a p) n -> p a n", p=128))
                ps = fps.tile([128, 512], F32, tag="f", name="psf")
                for i in range(NDT):
                    nc.tensor.matmul(ps[:], r(wup_t[:, 128 * i:128 * (i + 1)]),
                                     r(xaugT[:, QB * i:QB * (i + 1)]),
                                     start=(i == 0), stop=(i == NDT - 1))
                nc.scalar.activation(hT[:, QB * f:QB * (f + 1)], ps[:],
                                     AF.Gelu, bias=bup_sb[:, f:f + 1])

        # ============ FFN down + residual ============
        with tc.tile_pool(name="ysb", bufs=1) as yp, \
             tc.tile_pool(name="wdn", bufs=4) as wdnp, \
             tc.tile_pool(name="ops", bufs=8, space="PSUM") as ops:
            yT_sb = yp.tile([128, NDT * QB], F32, tag="yT")
            out_ps = [ops.tile([128, 512], F32, tag="o", name="o")
                      for _ in range(NDT)]
            for k in range(NFT):
                wdn_t = wdnp.tile([128, NDT * 128], F32R, tag="wdn", name="wdn_t")
                nc.sync.dma_start(wdn_t[:], t["wdownT"][128 * k:128 * (k + 1), :])
                for j in range(NDT):
                    nc.tensor.matmul(out_ps[j][:],
                                     r(wdn_t[:, 128 * j:128 * (j + 1)]),
                                     r(hT[:, QB * k:QB * (k + 1)]),
                                     start=(k == 0), stop=(k == NFT - 1))
            for j in range(NDT):
                nc.vector.scalar_tensor_tensor(
                    yT_sb[:, QB * j:QB * (j + 1)], out_ps[j][:],
                    cols["bdown_c"][:, j:j + 1], xt_own[:, QB * j:QB * (j + 1)],
                    op0=A.add, op1=A.add)
            nc.sync.dma_start(
                t["yT"][:].rearrange("(a p) n -> p a n", p=128),
                yT_sb[:].rearrange("p (a n) -> p a n", a=NDT))

    est.close()


# ---------------- host side ----------------

def prep_shared(inp):
    f = lambda a: np.ascontiguousarray(np.asarray(a, np.float32))
    bf = lambda a: np.ascontiguousarray(np.asarray(a, BF))
    cols8 = lambda v: np.ascontiguousarray(np.asarray(v, np.float32).reshape(NDT, 128).T)
    g1 = f(inp["g1"])
    b1 = f(inp["b1"])
    Wq, Wk, Wv = f(inp["Wq"]), f(inp["Wk"]), f(inp["Wv"])
    W = {}
    # LN1 affine folded into weights: W @ (xhat*g1 + b1) = (W*g1) @ xhat + W@b1
    W["wqT"] = bf((Wq.T * g1[:, None]) * 0.125)
    W["bq_c"] = cols8((f(inp["bq"]) + Wq @ b1) * 0.125)
    W["wkT"] = bf(Wk.T * g1[:, None])
    W["bk_c"] = cols8(f(inp["bk"]) + Wk @ b1)
    WvTg = Wv.T * g1[:, None]
    bv_eff = f(inp["bv"]) + Wv @ b1
    wv_ext = np.zeros((D, VW), np.float32)
    bv_ext = np.zeros((1, VW), np.float32)
    for h in range(H):
        wv_ext[:, 65 * h:65 * h + 64] = WvTg[:, 64 * h:64 * (h + 1)]
        bv_ext[0, 65 * h:65 * h + 64] = bv_eff[64 * h:64 * (h + 1)]
        bv_ext[0, 65 * h + 64] = 1.0
    W["wvT_ext"] = bf(wv_ext)
    W["bv_ext"] = bf(bv_ext)
    # Router folded: scores = [n1,ctx] @ (emb @ Ws).T  (+ emb@(bs + Ws1@b1))
    Ws = f(inp["Ws"])
    Ws1, Ws2 = Ws[:, :D], Ws[:, D:]
    emb = f(inp["neuron_emb"])
    W2x = (emb @ Ws1).T          # [D, NN]
    W["w2xT"] = np.ascontiguousarray(W2x * g1[:, None])
    W["w2cT"] = np.ascontiguousarray((emb @ Ws2).T)
    W["bias2"] = np.ascontiguousarray(
        (emb @ (f(inp["bs"]) + Ws1 @ b1)).reshape(1, NN))
    # info folded: info = w_dense @ (emb @ Wnp.T)
    W["embW"] = np.ascontiguousarray(emb @ f(inp["Wnp"]).T)
    W["wupT"] = np.ascontiguousarray(f(inp["Wup"]).T)
    W["wdownT"] = np.ascontiguousarray(f(inp["Wdown"]).T)
    W["b2np_c"] = cols8(f(inp["b2"]) + f(inp["bnp"]))
    W["g2_c"] = cols8(inp["g2"])
    W["bdown_c"] = cols8(inp["bdown"])
    W["bup_c"] = np.ascontiguousarray(f(inp["bup"]).reshape(NFT, 128).T)
    W["ones_col"] = np.ones((128, 1), np.float32)
    return W


_NC_CACHE = {}


def get_nc():
    if "nc" not in _NC_CACHE:
        _NC_CACHE["nc"] = build_program()
    return _NC_CACHE["nc"]


def make_in_maps(inputs):
    W = prep_shared(inputs)
    x = np.asarray(inputs["x"], np.float32)
    in_maps = []
    for c in range(8):
        b, qi = c // 4, c % 4
        q0 = qi * QB
        xT = np.ascontiguousarray(x[b].T)
        xTr = np.ascontiguousarray(np.concatenate([xT[:, q0:], xT[:, :q0]], axis=1))
        m = dict(W)
        m["xT"] = xTr
        in_maps.append(m)
    return in_maps


def kernel(**inputs):
    nc = get_nc()
    in_maps = make_in_maps(inputs)
    res = run_bass_kernel_spmd(nc, in_maps, core_ids=list(range(8)))
    x = np.asarray(inputs["x"])
    y = np.zeros((B, S, D), np.float32)
    for c in range(8):
        b, qi = c // 4, c % 4
        y[b, qi * QB:(qi + 1) * QB, :] = res.results[c]["yT"].T
    return y.astype(x.dtype, copy=False)


# revision 16
# speedup vs baseline: 1.2153x; 1.1275x over previous
"""Trainium2 Bass kernel for nn_DAWNLayer (moe_routing).

Sharding: data-parallel over (batch, sequence) across 8 cores — core c
handles batch c//4, query rows (c%4)*512..+512. K/V are computed
redundantly per core for the core's batch (full S=2048). All weights
replicated. Inputs are pre-transposed / pre-laid-out on the host; each
core's xT is cyclically rolled so its own query block is always cols
0:512 (keeps the SPMD program identical across cores; attention is
permutation-invariant along the key axis).

Device algorithm (activations kept transposed [feature, row] so matmul
contractions land on the partition dim):
  Per-512-token-chunk pipeline: LN stats -> xhat -> V+K for that chunk,
  with the next chunk's stats matmuls woven between the dense V/K
  matmuls (keeps the PE array active so the HAM clock gate stays at
  8/8). LN1 affine folded into Q/K/V weights; LN2 affine applied at the
  FFN-input eviction.
  Attention: even/odd heads interleaved so their K=64 score matmuls run
  concurrently in PE row groups 0-1 / 2-3 (tile_position derived from
  base partitions). Softmax without max-subtraction, denominator via a
  ones-column folded into V_ext, division folded into the ctx eviction.
  The attention phase is exp(ACT)-paced; the router-score xhat-half
  matmuls are woven into the PE idle slots.
  Router: scores = [n1,ctx] @ (emb@Ws).T computed directly (the query
  stage is folded into the weights on the host), dense top-8 masked
  softmax via vector.max + match_replace, then
  info = w_dense @ (emb@Wnp.T) as a dense matmul (Wnp folded on host).
  Attention path in bf16; router tail and FFN in fp32 data with
  float32r matmuls (full PE rate at N>=256).
"""

import os

os.environ.setdefault("MYCRO_LOCAL_CACHE", "1")

import numpy as np
import ml_dtypes

import concourse.bass as bass
import concourse.mybir as mybir
import concourse.tile as tile
from concourse import bacc
from concourse.bass_utils import run_bass_kernel_spmd
from concourse.masks import make_identity

dt = mybir.dt
BF = ml_dtypes.bfloat16

B, S, D = 2, 2048, 1024
DFF = 4096
H, DH = 16, 64
NN, K = 256, 8
EPS = 1e-5
QB = 512           # own query rows per core
NDT = D // 128     # 8 d-tiles
NTT = S // 128     # 16 t row-tiles
NFT = DFF // 128   # 32 dff tiles
NQS = QB // 128    # 4 q subtiles
VW = 65 * H        # 1040 V_ext width
RW = S - QB        # 1536 non-own rows

F32, F32R, BF16 = dt.float32, dt.float32r, dt.bfloat16


def r(ap):
    return ap.bitcast(F32R)


def build_program():
    nc = bacc.Bacc("TRN2", target_bir_lowering=False, debug=False, num_devices=8)

    def din(name, shape, dtype):
        return nc.dram_tensor(name, list(shape), dtype, kind="ExternalInput").ap()

    t = {}
    t["xT"] = din("xT", (D, S), F32R)
    t["wqT"] = din("wqT", (D, D), BF16)
    t["wkT"] = din("wkT", (D, D), BF16)
    t["wvT_ext"] = din("wvT_ext", (D, VW), BF16)
    t["bv_ext"] = din("bv_ext", (1, VW), BF16)
    t["w2xT"] = din("w2xT", (D, NN), F32R)      # (emb @ Ws1g).T, g1-folded
    t["w2cT"] = din("w2cT", (D, NN), F32R)      # (emb @ Ws2).T
    t["bias2"] = din("bias2", (1, NN), F32R)    # emb @ (bs + Ws1 @ b1)
    t["embW"] = din("embW", (NN, D), F32R)      # emb @ Wnp.T
    t["wupT"] = din("wupT", (D, DFF), F32R)
    t["wdownT"] = din("wdownT", (DFF, D), F32R)
    for nm in ("bq_c", "bk_c", "b2np_c", "bdown_c", "g2_c"):
        t[nm] = din(nm, (128, NDT), F32)
    t["bup_c"] = din("bup_c", (128, NFT), F32)
    t["ones_col"] = din("ones_col", (128, 1), F32R)
    t["yT"] = nc.dram_tensor("yT", [D, QB], F32, kind="ExternalOutput").ap()

    with tile.TileContext(nc) as tc:
        emit(tc, t)
    nc.compile()
    return nc


def emit(tc, t):
    from contextlib import ExitStack
    nc = tc.nc
    A = mybir.AluOpType
    AF = mybir.ActivationFunctionType
    X = mybir.AxisListType.X

    est = ExitStack()
    # ---- whole-kernel pools (left side) ----
    cp = est.enter_context(tc.tile_pool(name="consts", bufs=1))
    ap_ = est.enter_context(tc.tile_pool(name="poolA", bufs=1))
    bp = est.enter_context(tc.tile_pool(name="poolB", bufs=1))

    ones_col = cp.tile([128, 1], F32R, tag="ones_col")
    ones_row = cp.tile([1, 128], BF16, tag="ones_row")
    ones_row_f = cp.tile([1, 128], F32, tag="ones_row_f")
    eps_t = cp.tile([1, 1], F32, tag="eps_t")
    nc.vector.memset(eps_t[:], EPS)
    nc.sync.dma_start(ones_col[:], t["ones_col"])
    nc.vector.memset(ones_row[:], 1.0)
    nc.vector.memset(ones_row_f[:], 1.0)

    cols = {}
    for nm in ("bq_c", "bk_c", "b2np_c", "bdown_c", "g2_c"):
        cols[nm] = cp.tile([128, NDT], F32, tag=nm, name=nm)
        nc.sync.dma_start(cols[nm][:], t[nm])
    bup_sb = cp.tile([128, NFT], F32, tag="bup")
    nc.sync.dma_start(bup_sb[:], t["bup_c"])
    bv_sb = cp.tile([1, VW], BF16, tag="bv")
    nc.sync.dma_start(bv_sb[:], t["bv_ext"])
    bias2_sb = cp.tile([1, NN], F32R, tag="bias2")
    nc.sync.dma_start(bias2_sb[:], t["bias2"])

    xt_own = ap_.tile([128, NDT * QB], F32R, tag="xt_own")
    xh_f = bp.tile([128, NDT * QB], F32R, tag="xh_f")       # own xhat fp32

    nc.sync.dma_start(
        xt_own[:].rearrange("p (a n) -> p a n", a=NDT),
        t["xT"][:, 0:QB].rearrange("(a p) n -> p a n", p=128))

    # ---- right-side: bf16 xhat (phase A -> end of K) ----
    rstk = ExitStack()
    rp1 = rstk.enter_context(tc.tile_pool(name="xhbf", bufs=1, side="right"))
    xho_bf = rp1.tile([128, NDT * QB], BF16, tag="xho_bf")
    xhr_bf = rp1.tile([128, NDT * RW], BF16, tag="xhr_bf")

    def xhsl(i, c):  # xhat bf16 [d-tile i, t-chunk c of 512]
        if c == 0:
            return xho_bf[:, QB * i:QB * i + 512]
        return xhr_bf[:, RW * i + 512 * (c - 1):RW * i + 512 * c]

    def xhrow(i, tt):  # xhat bf16 [d-tile i, 128 rows of row-tile tt]
        if tt < 4:
            return xho_bf[:, QB * i + 128 * tt:QB * i + 128 * (tt + 1)]
        return xhr_bf[:, RW * i + 128 * (tt - 4):RW * i + 128 * (tt - 3)]

    # ====== Phase A: per-chunk LN stats -> xhat -> V (stats woven) ======
    attn_stack = ExitStack()
    vxp = attn_stack.enter_context(tc.tile_pool(name="vxp", bufs=1))
    Vx = vxp.tile([128, NTT * VW], BF16, tag="Vx")

    with tc.tile_pool(name="p1sb", bufs=1) as p1, \
         tc.tile_pool(name="rows", bufs=12) as rows, \
         tc.tile_pool(name="bcp", bufs=2) as bcp, \
         tc.tile_pool(name="xsqp", bufs=2) as xsqp, \
         tc.tile_pool(name="statps", bufs=1, space="PSUM") as statps, \
         tc.tile_pool(name="vps", bufs=2, space="PSUM") as vps:
        xt_rest = p1.tile([128, NDT * RW], F32R, tag="xt_rest")
        for cc in range(3):
            nc.sync.dma_start(
                xt_rest[:].rearrange("p (a n) -> p a n", a=NDT)[
                    :, :, 512 * cc:512 * (cc + 1)],
                t["xT"][:, QB + 512 * cc:QB + 512 * (cc + 1)].rearrange(
                    "(a p) n -> p a n", p=128))
        wv_all = p1.tile([128, NDT * VW], BF16, tag="wv_all")
        for c4 in range(4):
            nc.sync.dma_start(
                wv_all[:].rearrange("p (a n) -> p a n", a=NDT)[:, :, 260 * c4:260 * (c4 + 1)],
                t["wvT_ext"][:, 260 * c4:260 * (c4 + 1)].rearrange(
                    "(a p) n -> p a n", p=128))

        def xsl(i, c):
            if c == 0:
                return xt_own[:, QB * i:QB * i + 512]
            return xt_rest[:, RW * i + 512 * (c - 1):RW * i + 512 * c]

        # stats(c): one [1,1024] PSUM tile; [0:512]=sum x, [512:1024]=sum x^2
        stat_ps = {}

        def emit_stats(c, i):
            if i == 0:
                stat_ps[c] = statps.tile([1, 1024], F32, tag="stat", name="stat")
            xsq = xsqp.tile([128, 512], F32R, tag="xsq", name="xsq")
            nc.scalar.activation(xsq[:], xsl(i, c), AF.Square)
            nc.tensor.matmul(stat_ps[c][:, 0:512], r(ones_col[:]), r(xsl(i, c)),
                             start=(i == 0), stop=(i == NDT - 1))
            nc.tensor.matmul(stat_ps[c][:, 512:1024], r(ones_col[:]), r(xsq[:]),
                             start=(i == 0), stop=(i == NDT - 1))

        def finish_stats(c):
            # -> rstd_bc, mhat_bc broadcast tiles for this chunk
            ps = stat_ps.pop(c)
            m_c = rows.tile([1, 512], F32, tag="rows", name="m_c")
            v_c = rows.tile([1, 512], F32, tag="rows", name="v_c")
            nc.scalar.activation(m_c[:], ps[:, 0:512], AF.Copy, scale=1.0 / D)
            nc.scalar.activation(v_c[:], ps[:, 512:1024], AF.Copy, scale=1.0 / D)
            msq = rows.tile([1, 512], F32, tag="rows", name="msq")
            nc.vector.tensor_mul(msq[:], m_c[:], m_c[:])
            nc.vector.tensor_sub(v_c[:], v_c[:], msq[:])
            sdev = rows.tile([1, 512], F32, tag="rows", name="sdev")
            nc.scalar.activation(sdev[:], v_c[:], AF.Sqrt, bias=eps_t[:])
            rstd_c = rows.tile([1, 512], F32, tag="rows", name="rstd_c")
            nc.vector.reciprocal(rstd_c[:], sdev[:])
            mhat_c = rows.tile([1, 512], F32, tag="rows", name="mhat_c")
            nc.vector.tensor_mul(mhat_c[:], m_c[:], rstd_c[:])
            rstd_bc = bcp.tile([128, 512], F32, tag="rstd_bc", name="rstd_bc")
            mhat_bc = bcp.tile([128, 512], F32, tag="mhat_bc", name="mhat_bc")
            nc.gpsimd.partition_broadcast(rstd_bc[:], rstd_c[:])
            nc.gpsimd.partition_broadcast(mhat_bc[:], mhat_c[:])
            return rstd_bc, mhat_bc

        def emit_xhat(c, rstd_bc, mhat_bc):
            for i in range(NDT):
                if c == 0:
                    o = xh_f[:, QB * i:QB * (i + 1)]
                    nc.vector.tensor_mul(o, xt_own[:, QB * i:QB * i + 512],
                                         rstd_bc[:])
                    nc.vector.tensor_sub(o, o, mhat_bc[:])
                    nc.vector.tensor_copy(xho_bf[:, QB * i:QB * (i + 1)], o)
                else:
                    rr = xt_rest[:, RW * i + 512 * (c - 1):RW * i + 512 * c]
                    nc.vector.tensor_mul(rr, rr, rstd_bc[:])
                    nc.vector.tensor_sub(
                        xhr_bf[:, RW * i + 512 * (c - 1):RW * i + 512 * c],
                        rr, mhat_bc[:])

        def emit_v_tt(tt):
            for half in range(2):
                ps = vps.tile([128, 1024], F32, tag="v", name="psv")
                for cc in range(2):
                    c4 = 2 * half + cc
                    dst = ps[:, 512 * cc:512 * cc + 260]
                    for i in range(NDT):
                        nc.tensor.matmul(
                            dst, xhrow(i, tt),
                            wv_all[:, VW * i + 260 * c4:VW * i + 260 * (c4 + 1)],
                            start=(i == 0), stop=False)
                    nc.tensor.matmul(dst, ones_row[:],
                                     bv_sb[:, 260 * c4:260 * (c4 + 1)],
                                     start=False, stop=True)
                src2 = ps[:].rearrange("p (c n) -> p c n", c=2)[:, :, 0:260]
                dst2 = Vx[:, VW * tt + 520 * half:VW * tt + 520 * (half + 1)]
                nc.scalar.activation(
                    dst2.rearrange("p (c n) -> p c n", c=2), src2, AF.Copy)

        # chunk 0 stats up-front
        for i in range(NDT):
            emit_stats(0, i)
        for c in range(4):
            rstd_bc, mhat_bc = finish_stats(c)
            emit_xhat(c, rstd_bc, mhat_bc)
            # V for this chunk's 4 row-tiles; weave next chunk's stats
            for k4, tt in enumerate(range(4 * c, 4 * (c + 1))):
                emit_v_tt(tt)
                if c < 3:
                    emit_stats(c + 1, 2 * k4)
                    emit_stats(c + 1, 2 * k4 + 1)

    # ============ Q, K(0); K(1..7) woven into attention ============
    c2 = attn_stack.enter_context(tc.tile_pool(name="attn", bufs=1))
    Kt = c2.tile([128, NDT * S], BF16, tag="Kt")
    Qt = c2.tile([128, NDT * QB], BF16, tag="Qt")

    wkq_stack = ExitStack()
    wkq = wkq_stack.enter_context(tc.tile_pool(name="wkq", bufs=3))
    kqps_pool = wkq_stack.enter_context(
        tc.tile_pool(name="kqps", bufs=1, space="PSUM"))

    def load_wk(j):
        wk_t = wkq.tile([128, NDT * 128], BF16, tag="wkq", name="wk_t")
        nc.sync.dma_start(
            wk_t[:].rearrange("p (a n) -> p a n", a=NDT),
            t["wkT"][:, 128 * j:128 * (j + 1)].rearrange("(a p) n -> p a n", p=128))
        return wk_t

    k_ps = {}

    def emit_k_mm(j, wk_t, c, i):
        if i == 0:
            k_ps[j] = kqps_pool.tile([128, 512], F32, tag="kq", name="psk")
        nc.tensor.matmul(k_ps[j][:], wk_t[:, 128 * i:128 * (i + 1)],
                         xhsl(i, c), start=(i == 0), stop=(i == NDT - 1))
        if i == NDT - 1:
            # K eviction on DVE (per-partition bias add + bf16 cast)
            nc.vector.tensor_scalar(
                Kt[:, S * j + 512 * c:S * j + 512 * (c + 1)],
                k_ps.pop(j)[:], cols["bk_c"][:, j:j + 1], None, A.add)

    with tc.tile_pool(name="qps", bufs=2, space="PSUM") as qps:
        for j in range(NDT):
            wq_t = wkq.tile([128, NDT * 128], BF16, tag="wkq", name="wq_t")
            nc.sync.dma_start(
                wq_t[:].rearrange("p (a n) -> p a n", a=NDT),
                t["wqT"][:, 128 * j:128 * (j + 1)].rearrange("(a p) n -> p a n", p=128))
            ps = qps.tile([128, 512], F32, tag="q", name="psq")
            for i in range(NDT):
                nc.tensor.matmul(ps[:], wq_t[:, 128 * i:128 * (i + 1)],
                                 xho_bf[:, QB * i:QB * (i + 1)],
                                 start=(i == 0), stop=(i == NDT - 1))
            nc.scalar.activation(Qt[:, QB * j:QB * (j + 1)], ps[:],
                                 AF.Identity, bias=cols["bq_c"][:, j:j + 1])
        wk0 = load_wk(0)
        for c in range(4):
            for i in range(NDT):
                emit_k_mm(0, wk0, c, i)

    # ---- right-side: router tensors (attention -> xaug) ----
    re_ = rstk.enter_context(tc.tile_pool(name="router", bufs=1, side="right"))
    ctxT = re_.tile([128, NDT * QB], F32R, tag="ctxT")
    snn_sb = re_.tile([128, NQS * NN], F32, tag="snn")
    w_all = re_.tile([128, NQS * NN], F32, tag="w_all")
    wT_sb = re_.tile([128, 2 * QB], F32R, tag="wT")

    # ============ Phase B: attention (paired heads, K weave) ============
    with tc.tile_pool(name="expp", bufs=4) as expp, \
         tc.tile_pool(name="denp", bufs=2) as denp, \
         tc.tile_pool(name="stps", bufs=2, space="PSUM") as stps, \
         tc.tile_pool(name="ctxps", bufs=3, space="PSUM") as ctxps:

        def evict_ctx(h, ctx_ps):
            j, po = h // 2, 64 * (h % 2)
            den = denp.tile([1, 512], F32, tag="den", name="den")
            nc.scalar.activation(den[:], ctx_ps[64:65, :], AF.Copy)
            rec = denp.tile([1, 512], F32, tag="rec", name="rec")
            nc.vector.reciprocal(rec[:], den[:])
            rbc = denp.tile([64, 512], F32, tag="rbc", name="rbc")
            nc.gpsimd.partition_broadcast(rbc[:], rec[:])
            nc.vector.tensor_mul(ctxT[po:po + 64, QB * j:QB * (j + 1)],
                                 ctx_ps[0:64, :], rbc[:])

        for hp in range(NDT):
            h_e, h_o = 2 * hp, 2 * hp + 1
            ctx_e = ctxps.tile([65, 512], F32, tag="ctx", name="ctx_e")
            ctx_o = ctxps.tile([65, 512], F32, tag="ctx", name="ctx_o")
            wk_t = load_wk(hp + 1) if hp + 1 < NDT else None
            eU_q = []  # pending (eU, tt) for AV, one half-slot of lag
            for tt in range(NTT):
                # scores: both parities into one st tile -> one exp gate;
                # adjacent MMs hit PE row groups 0-1 / 2-3 concurrently
                st = stps.tile([128, 1024], F32, tag="st", name="st")
                ksl = Kt[:, S * hp + 128 * tt:S * hp + 128 * (tt + 1)]
                qsl = Qt[:, QB * hp:QB * (hp + 1)]
                nc.tensor.matmul(st[:, 0:512], ksl[0:64, :], qsl[0:64, :],
                                 start=True, stop=True)
                nc.tensor.matmul(st[:, 512:1024], ksl[64:128, :], qsl[64:128, :],
                                 start=True, stop=True)
                eU = expp.tile([128, 1024], BF16, tag="eU", name="eU")
                nc.scalar.activation(eU[:], st[:], AF.Exp)
                # weave 2 K-projection matmuls (full-array work keeps HAM warm)
                if wk_t is not None:
                    for kk in (2 * tt, 2 * tt + 1):
                        emit_k_mm(hp + 1, wk_t, kk // NDT, kk % NDT)
                eU_q.append((eU, tt))
                if len(eU_q) > 1:
                    eUp, ttp = eU_q.pop(0)
                    nc.tensor.matmul(ctx_e[:],
                                     Vx[:, VW * ttp + 65 * h_e:VW * ttp + 65 * (h_e + 1)],
                                     eUp[:, 0:512],
                                     start=(ttp == 0), stop=(ttp == NTT - 1))
                    nc.tensor.matmul(ctx_o[:],
                                     Vx[:, VW * ttp + 65 * h_o:VW * ttp + 65 * (h_o + 1)],
                                     eUp[:, 512:1024],
                                     start=(ttp == 0), stop=(ttp == NTT - 1))
            eUp, ttp = eU_q.pop(0)
            nc.tensor.matmul(ctx_e[:],
                             Vx[:, VW * ttp + 65 * h_e:VW * ttp + 65 * (h_e + 1)],
                             eUp[:, 0:512],
                             start=(ttp == 0), stop=(ttp == NTT - 1))
            nc.tensor.matmul(ctx_o[:],
                             Vx[:, VW * ttp + 65 * h_o:VW * ttp + 65 * (h_o + 1)],
                             eUp[:, 512:1024],
                             start=(ttp == 0), stop=(ttp == NTT - 1))
            evict_ctx(h_e, ctx_e)
            evict_ctx(h_o, ctx_o)

    wkq_stack.close()
    attn_stack.close()  # free Kt/Vx/Qt
    ap2 = est.enter_context(tc.tile_pool(name="poolA2", bufs=1))
    xaugT = ap2.tile([128, NDT * QB], F32R, tag="xaugT")
    ident = ap2.tile([128, 128], F32, tag="ident")
    make_identity(nc, ident[:])

    # ======= snn: [n1,ctx] @ (emb@Ws).T + emb@bs =======
    snn_stack = ExitStack()
    w2p = snn_stack.enter_context(tc.tile_pool(name="w2p", bufs=1))
    snnps_pool = snn_stack.enter_context(
        tc.tile_pool(name="snnps", bufs=1, space="PSUM"))
    snn_ps = snnps_pool.tile([128, NQS * NN], F32, tag="snnp")
    w2x_sb = w2p.tile([128, NDT * NN], F32R, tag="w2x")
    w2c_sb = w2p.tile([128, NDT * NN], F32R, tag="w2c")
    nc.sync.dma_start(
        w2x_sb[:].rearrange("p (a n) -> p a n", a=NDT),
        t["w2xT"][:].rearrange("(a p) n -> p a n", p=128))
    nc.sync.dma_start(
        w2c_sb[:].rearrange("p (a n) -> p a n", a=NDT),
        t["w2cT"][:].rearrange("(a p) n -> p a n", p=128))
    for qs in range(NQS):
        for i in range(NDT):
            nc.tensor.matmul(
                snn_ps[:, NN * qs:NN * (qs + 1)],
                xh_f[:, QB * i + 128 * qs:QB * i + 128 * (qs + 1)].bitcast(F32R),
                r(w2x_sb[:, NN * i:NN * (i + 1)]),
                start=(i == 0), stop=False)
        nc.tensor.matmul(
            snn_ps[:, NN * qs:NN * (qs + 1)],
            r(ones_row_f[:, 0:128]), bias2_sb[:],
            start=False, stop=False)
        for i in range(NDT):
            nc.tensor.matmul(
                snn_ps[:, NN * qs:NN * (qs + 1)],
                r(ctxT[:, QB * i + 128 * qs:QB * i + 128 * (qs + 1)]),
                r(w2c_sb[:, NN * i:NN * (i + 1)]),
                start=False, stop=(i == NDT - 1))
        nc.scalar.activation(snn_sb[:, NN * qs:NN * (qs + 1)],
                             snn_ps[:, NN * qs:NN * (qs + 1)], AF.Copy)

    snn_stack.close()

    # ======= router top-8, info, xaug =======
    with tc.tile_pool(name="rtp", bufs=2) as rtp, \
         tc.tile_pool(name="aps", bufs=3, space="PSUM") as aps, \
         tc.tile_pool(name="trps", bufs=2, space="PSUM") as trps:
        for qs in range(NQS):
            snn = snn_sb[:, NN * qs:NN * (qs + 1)]
            t8 = rtp.tile([128, 8], F32, tag="t8", name="t8")
            nc.vector.max(t8[:], snn)
            nmx = rtp.tile([128, 1], F32, tag="nmx", name="nmx")
            nc.vector.tensor_scalar(nmx[:], t8[:, 0:1], -1.0, None, A.mult)
            snz = rtp.tile([128, NN], F32, tag="snz", name="snz")
            nc.vector.match_replace(out=snz[:], in_to_replace=t8[:],
                                    in_values=snn, imm_value=-1e30)
            e = rtp.tile([128, NN], F32, tag="e", name="e")
            nc.scalar.activation(e[:], snn, AF.Exp, bias=nmx[:])
            mask = rtp.tile([128, NN], F32, tag="mask", name="mask")
            nc.vector.tensor_tensor(mask[:], snn, snz[:], A.not_equal)
            wu = rtp.tile([128, NN], F32, tag="wu", name="wu")
            nc.vector.tensor_mul(wu[:], e[:], mask[:])
            ssum = rtp.tile([128, 1], F32, tag="ssum", name="ssum")
            nc.vector.tensor_reduce(ssum[:], wu[:], X, A.add)
            rcp = rtp.tile([128, 1], F32, tag="rcp", name="rcp")
            nc.vector.reciprocal(rcp[:], ssum[:])
            nc.vector.tensor_scalar(w_all[:, NN * qs:NN * (qs + 1)], wu[:],
                                    rcp[:], None, A.mult)

        for qs in range(NQS):
            for n in range(2):
                ps = trps.tile([128, 128], F32, tag="tr", name="pstr")
                nc.tensor.transpose(
                    ps[:],
                    w_all[:, NN * qs + 128 * n:NN * qs + 128 * (n + 1)],
                    ident[:])
                nc.scalar.activation(
                    wT_sb[:, QB * n + 128 * qs:QB * n + 128 * (qs + 1)],
                    ps[:], AF.Copy)

        with tc.tile_pool(name="embp", bufs=1) as embp:
            emb_sb = embp.tile([128, 2 * D], F32R, tag="emb")
            for n in range(2):
                nc.sync.dma_start(emb_sb[:, D * n:D * (n + 1)],
                                  t["embW"][128 * n:128 * (n + 1), :])
            for j in range(NDT):
                ps = aps.tile([128, 512], F32, tag="a", name="psinfo")
                for n in range(2):
                    nc.tensor.matmul(
                        ps[:],
                        r(emb_sb[:, D * n + 128 * j:D * n + 128 * (j + 1)]),
                        r(wT_sb[:, QB * n:QB * (n + 1)]),
                        start=(n == 0), stop=(n == 1))
                xa = xaugT[:, QB * j:QB * (j + 1)]
                # n2 + bnp = xhat*g2 + (b2 + bnp)
                nc.vector.tensor_scalar(xa, xh_f[:, QB * j:QB * (j + 1)],
                                        cols["g2_c"][:, j:j + 1],
                                        cols["b2np_c"][:, j:j + 1], A.mult, A.add)
                nc.vector.tensor_add(xa, ps[:], xa)

    rstk.close()  # free router tensors

    # ============ FFN up (gelu) ============
    with tc.tile_pool(name="hsb", bufs=1) as hp:
        hT = hp.tile([128, NFT * QB], F32R, tag="hT")
        with tc.tile_pool(name="wup", bufs=4) as wupp, \
             tc.tile_pool(name="fps", bufs=2, space="PSUM") as fps:
            for f in range(NFT):
                wup_t = wupp.tile([128, NDT * 128], F32R, tag="wup", name="wup_t")
                nc.sync.dma_start(
                    wup_t[:].rearrange("p (a n) -> p a n", a=NDT),
                    t["wupT"][:, 128 * f:128 * (f + 1)].rearrange("(a p) n -> p a n", p=128))
                ps = fps.tile([128, 512], F32, tag="f", name="psf")
                for i in range(NDT):
                    nc.tensor.matmul(ps[:], r(wup_t[:, 128 * i:128 * (i + 1)]),
                                     r(xaugT[:, QB * i:QB * (i + 1)]),
                                     start=(i == 0), stop=(i == NDT - 1))
                nc.scalar.activation(hT[:, QB * f:QB * (f + 1)], ps[:],
                                     AF.Gelu, bias=bup_sb[:, f:f + 1])

        # ============ FFN down + residual ============
        with tc.tile_pool(name="ysb", bufs=1) as yp, \
             tc.tile_pool(name="wdn", bufs=4) as wdnp, \
             tc.tile_pool(name="ops", bufs=8, space="PSUM") as ops:
            yT_sb = yp.tile([128, NDT * QB], F32, tag="yT")
            out_ps = [ops.tile([128, 512], F32, tag="o", name="o")
                      for _ in range(NDT)]
            for k in range(NFT):
                wdn_t = wdnp.tile([128, NDT * 128], F32R, tag="wdn", name="wdn_t")
                nc.sync.dma_start(wdn_t[:], t["wdownT"][128 * k:128 * (k + 1), :])
                for j in range(NDT):
                    nc.tensor.matmul(out_ps[j][:],
                                     r(wdn_t[:, 128 * j:128 * (j + 1)]),
                                     r(hT[:, QB * k:QB * (k + 1)]),
                                     start=(k == 0), stop=(k == NFT - 1))
            for j in range(NDT):
                nc.vector.scalar_tensor_tensor(
                    yT_sb[:, QB * j:QB * (j + 1)], out_ps[j][:],
                    cols["bdown_c"][:, j:j + 1], xt_own[:, QB * j:QB * (j + 1)],
                    op0=A.add, op1=A.add)
            nc.sync.dma_start(
                t["yT"][:].rearrange("(a p) n -> p a n", p=128),
                yT_sb[:].rearrange("p (a n) -> p a n", a=NDT))

    est.close()


# ---------------- host side ----------------

def prep_shared(inp):
    f = lambda a: np.ascontiguousarray(np.asarray(a, np.float32))
    bf = lambda a: np.ascontiguousarray(np.asarray(a, BF))
    cols8 = lambda v: np.ascontiguousarray(np.asarray(v, np.float32).reshape(NDT, 128).T)
    g1 = f(inp["g1"])
    b1 = f(inp["b1"])
    Wq, Wk, Wv = f(inp["Wq"]), f(inp["Wk"]), f(inp["Wv"])
    W = {}
    # LN1 affine folded into weights: W @ (xhat*g1 + b1) = (W*g1) @ xhat + W@b1
    W["wqT"] = bf((Wq.T * g1[:, None]) * 0.125)
    W["bq_c"] = cols8((f(inp["bq"]) + Wq @ b1) * 0.125)
    W["wkT"] = bf(Wk.T * g1[:, None])
    W["bk_c"] = cols8(f(inp["bk"]) + Wk @ b1)
    WvTg = Wv.T * g1[:, None]
    bv_eff = f(inp["bv"]) + Wv @ b1
    wv_ext = np.zeros((D, VW), np.float32)
    bv_ext = np.zeros((1, VW), np.float32)
    for h in range(H):
        wv_ext[:, 65 * h:65 * h + 64] = WvTg[:, 64 * h:64 * (h + 1)]
        bv_ext[0, 65 * h:65 * h + 64] = bv_eff[64 * h:64 * (h + 1)]
        bv_ext[0, 65 * h + 64] = 1.0
    W["wvT_ext"] = bf(wv_ext)
    W["bv_ext"] = bf(bv_ext)
    # Router folded: scores = [n1,ctx] @ (emb @ Ws).T  (+ emb@(bs + Ws1@b1))
    Ws = f(inp["Ws"])
    Ws1, Ws2 = Ws[:, :D], Ws[:, D:]
    emb = f(inp["neuron_emb"])
    W2x = (emb @ Ws1).T          # [D, NN]
    W["w2xT"] = np.ascontiguousarray(W2x * g1[:, None])
    W["w2cT"] = np.ascontiguousarray((emb @ Ws2).T)
    W["bias2"] = np.ascontiguousarray(
        (emb @ (f(inp["bs"]) + Ws1 @ b1)).reshape(1, NN))
    # info folded: info = w_dense @ (emb @ Wnp.T)
    W["embW"] = np.ascontiguousarray(emb @ f(inp["Wnp"]).T)
    W["wupT"] = np.ascontiguousarray(f(inp["Wup"]).T)
    W["wdownT"] = np.ascontiguousarray(f(inp["Wdown"]).T)
    W["b2np_c"] = cols8(f(inp["b2"]) + f(inp["bnp"]))
    W["g2_c"] = cols8(inp["g2"])
    W["bdown_c"] = cols8(inp["bdown"])
    W["bup_c"] = np.ascontiguousarray(f(inp["bup"]).reshape(NFT, 128).T)
    W["ones_col"] = np.ones((128, 1), np.float32)
    return W


_NC_CACHE = {}


def get_nc():
    if "nc" not in _NC_CACHE:
        _NC_CACHE["nc"] = build_program()
    return _NC_CACHE["nc"]


def make_in_maps(inputs):
    W = prep_shared(inputs)
    x = np.asarray(inputs["x"], np.float32)
    in_maps = []
    for c in range(8):
        b, qi = c // 4, c % 4
        q0 = qi * QB
        xT = np.ascontiguousarray(x[b].T)
        xTr = np.ascontiguousarray(np.concatenate([xT[:, q0:], xT[:, :q0]], axis=1))
        m = dict(W)
        m["xT"] = xTr
        in_maps.append(m)
    return in_maps


def kernel(**inputs):
    nc = get_nc()
    in_maps = make_in_maps(inputs)
    res = run_bass_kernel_spmd(nc, in_maps, core_ids=list(range(8)))
    x = np.asarray(inputs["x"])
    y = np.zeros((B, S, D), np.float32)
    for c in range(8):
        b, qi = c // 4, c % 4
        y[b, qi * QB:(qi + 1) * QB, :] = res.results[c]["yT"].T
    return y.astype(x.dtype, copy=False)
